# revision 54
# baseline (speedup 1.0000x reference)
"""Causal multi-head self-attention (QK-RMSNorm + tanh softcap) on 8 trn2 cores.

Problem (hardcoded): x [2, 2048, 1024], w_q/w_k/w_v/w_o [1024, 1024] fp32,
H=16 heads, dk=64, softcap 50, causal, out = softmax-attn @ w_o.T.

Sharding: head-parallel. Core c owns heads {2c, 2c+1} (128 local dims):
  - w_q/w_k/w_v sliced by rows -> per-core [128, 1024]; host pre-transposes.
  - w_o sliced by columns -> per-core [1024, 128]; host pre-transposes.
  - x is replicated (host pre-transposed to xT [1024, 4096], bf16).
  - Each core emits a full-shape bf16 partial output [4096, 1024]; host sums.

Numerics: the tanh softcap is dropped (|logits| <= 8 by Cauchy-Schwarz after
QK RMS norm, so tanh(s/50)*50 ~ s to ~2e-3 relative; measured end-to-end
error vs the fp32 reference is ~3.5e-3 against a 2e-2 gate, including the
bf16 input/output quantization). exp needs no running max (logits bounded).

On-core pipeline per 512-token tile (matmuls keyed on the bf16/f32r moving
operand run at 1 cycle/row):
  A) q/k: ps = wT.T @ x (8 k-chunk matmuls, bf16 in, f32 PSUM), staged to
     resident f32r qTn/kTn; per-head sumsq via eye2 matmul of sq = ACT
     Square(ps); rstd = 1/sqrt(ss) via a packed quake-Newton rsqrt
     (PE-transpose [2,512] rows into [128,16], seed on DVE, Newton iters on
     the otherwise-idle Pool engine, transpose back); the x8 RMS factor is
     folded into the eyeT8q/k broadcast matmuls; in-place normalize on DVE.
     v: projected directly into [token, dim] layout by 32 small matmuls
     (lhsT = x chunk), staged into vaug with ones columns for the softmax
     denominator. Fronts (through quake) and backs (unpack + normalize)
     are software-pipelined one tile apart.
  B) per (batch, head, 512-query block): scoresT[j, i] = k.T @ q blocks into
     [128, 1024] PSUM, exp straight off PSUM (scale 1/8) into f32r u tiles
     (diagonal trapezoid first so Pool mask latency stays off the PV tail),
     causal via block skip + triangular mask multiply (Pool), PV accumulate
     with v stationary (denominator rides along as row 64), normalize via
     reciprocal + K=1 ones matmul broadcast + DVE multiply. Output
     projections lag their attention core by one block.
  C) out[t, :] = ytt.T @ w_oT per 128-token block; PSUM staged to a bf16
     [128, 4, 1024] tile (DVE, with ACT taking a share; alternating on the
     drain blocks) and DMAd per 128-token row block.
"""

import sys

for _p in ("/opt/trn_rl_repo",):
    if _p not in sys.path:
        sys.path.insert(0, _p)

import numpy as np

import concourse.bacc as bacc
import concourse.tile as tile
from concourse import mybir
from concourse.bass_utils import run_bass_kernel_spmd

F32 = mybir.dt.float32
F32R = mybir.dt.float32r
BF16 = mybir.dt.bfloat16
AF = mybir.ActivationFunctionType
ALU = mybir.AluOpType

B, S, D = 2, 2048, 1024
H, DK = 16, 64
NCORES = 8
HLOC = H // NCORES          # 2 heads per core
MLOC = HLOC * DK            # 128 local head dims
T = B * S                   # 4096 tokens

TT = 512                    # token tile (phase A, also query i-block)
NTB = S // TT               # 4 token tiles per batch
JB = 128                    # key j-block
NJB = S // JB               # 16 j-blocks per batch
ISQDK = 1.0 / 8.0           # 1/sqrt(64)


def build_kernel():
    nc = bacc.Bacc("TRN2", target_bir_lowering=False, debug=False)

    xT = nc.dram_tensor("xT", [D, T], BF16, kind="ExternalInput")
    wqT = nc.dram_tensor("wqT", [D, MLOC], BF16, kind="ExternalInput")
    wkT = nc.dram_tensor("wkT", [D, MLOC], BF16, kind="ExternalInput")
    wvT = nc.dram_tensor("wvT", [D, MLOC], BF16, kind="ExternalInput")
    woT = nc.dram_tensor("woT", [MLOC, D], BF16, kind="ExternalInput")
    out = nc.dram_tensor("out", [T, D], BF16, kind="ExternalOutput")

    xT3 = xT.ap().rearrange("(o p) t -> p o t", p=128)      # [128, 8, 4096]
    out3 = out.ap().rearrange("(blk p) d -> p blk d", p=128)  # [128, 32, 1024]

    with tile.TileContext(nc) as tc:
        _emit(nc, tc, xT3, wqT, wkT, wvT, woT, out3)

    nc.compile()
    return nc


def _emit(nc, tc, xT3, wqT, wkT, wvT, woT, out3):
    from contextlib import ExitStack

    ctx = ExitStack()
    with ctx:
        cn = ctx.enter_context(tc.tile_pool(name="cn", bufs=1))
        # small staging tiles for the packed quake rsqrt chain
        nwt = ctx.enter_context(tc.tile_pool(name="nwt", bufs=8))
        xload = ctx.enter_context(tc.tile_pool(name="xload", bufs=3))
        wpool = ctx.enter_context(tc.tile_pool(name="wpool", bufs=1))
        qk_res = ctx.enter_context(tc.tile_pool(name="qk_res", bufs=1))
        sqp = ctx.enter_context(tc.tile_pool(name="sqp", bufs=3))
        rsp = ctx.enter_context(tc.tile_pool(name="rsp", bufs=4))
        up = ctx.enter_context(tc.tile_pool(name="up", bufs=6))
        ytn = ctx.enter_context(tc.tile_pool(name="ytn", bufs=3))
        osp = ctx.enter_context(tc.tile_pool(name="osp", bufs=2))
        # PSUM: sc 2x[128,1024] (4 banks) + yt 2x[65,512] (2) + mm 2x[128,512]
        # (2) = 8 banks. The whole rstd chain runs through PE transposes and
        # engine ops (no DMA hops), so its mm-slot reuse drains fast enough
        # to share the single mm tag with proj/outproj tiles.
        scp = ctx.enter_context(tc.tile_pool(name="scp", bufs=2, space="PSUM"))
        ytp = ctx.enter_context(tc.tile_pool(name="ytp", bufs=2, space="PSUM"))
        mmp = ctx.enter_context(tc.tile_pool(name="mmp", bufs=2, space="PSUM"))

        # ---- constants ----
        def rounded(name, f32_tile):
            t = cn.tile(list(f32_tile.shape), F32R, tag=name + "_r", name=name + "_r")
            nc.vector.tensor_copy(t, f32_tile)
            return t

        eye2_f = cn.tile([128, 2], F32, tag="eye2_f")
        nc.vector.memset(eye2_f, 0.0)
        nc.vector.memset(eye2_f[0:64, 0:1], 1.0)
        nc.vector.memset(eye2_f[64:128, 1:2], 1.0)
        eye2 = rounded("eye2", eye2_f)

        ident = cn.tile([128, 128], F32, tag="ident")
        nc.vector.memset(ident, 1.0)
        nc.gpsimd.affine_select(
            out=ident, in_=ident, pattern=[[1, 128]],
            compare_op=ALU.is_equal, fill=0.0, base=0, channel_multiplier=-1,
        )

        ident4 = cn.tile([4, 4], F32, tag="ident4")
        nc.vector.memset(ident4, 1.0)
        nc.gpsimd.affine_select(
            out=ident4, in_=ident4, pattern=[[1, 4]],
            compare_op=ALU.is_equal, fill=0.0, base=0, channel_multiplier=-1,
        )

        # eyeT8q[p, f] = 8 iff 0 <= f - 64p < 64 (rows 2,3 fall out as zero):
        # broadcasts rt rows 0:2 (q rstd per head) across their 64 partitions,
        # folding in the x8 RMS factor (rstd = 1/sqrt(sumsq); rms norm needs
        # 8/sqrt(sumsq)). eyeT8k does the same for rt rows 2:4 (k rstd).
        eyeT8q_f = cn.tile([4, 128], F32, tag="eyeT8q_f")
        nc.vector.memset(eyeT8q_f, 8.0)
        nc.gpsimd.affine_select(
            out=eyeT8q_f, in_=eyeT8q_f, pattern=[[1, 128]],
            compare_op=ALU.is_ge, fill=0.0, base=0, channel_multiplier=-64,
        )
        nc.gpsimd.affine_select(
            out=eyeT8q_f, in_=eyeT8q_f, pattern=[[-1, 128]],
            compare_op=ALU.is_ge, fill=0.0, base=63, channel_multiplier=64,
        )
        eyeT8q = rounded("eyeT8q", eyeT8q_f)

        eyeT8k_f = cn.tile([4, 128], F32, tag="eyeT8k_f")
        nc.vector.memset(eyeT8k_f, 8.0)
        nc.gpsimd.affine_select(
            out=eyeT8k_f, in_=eyeT8k_f, pattern=[[1, 128]],
            compare_op=ALU.is_ge, fill=0.0, base=128, channel_multiplier=-64,
        )
        nc.gpsimd.affine_select(
            out=eyeT8k_f, in_=eyeT8k_f, pattern=[[-1, 128]],
            compare_op=ALU.is_ge, fill=0.0, base=-65, channel_multiplier=64,
        )
        eyeT8k = rounded("eyeT8k", eyeT8k_f)

        ones1_f = cn.tile([1, 64], F32, tag="ones1_f")
        nc.vector.memset(ones1_f, 1.0)
        ones1 = rounded("ones1", ones1_f)

        # tri128[p, f] = 1 if f >= p else 0  (keep i>=j in [j, i] tiles)
        tri_f = cn.tile([128, 128], F32, tag="tri_f")
        nc.vector.memset(tri_f, 1.0)
        nc.gpsimd.affine_select(
            out=tri_f, in_=tri_f, pattern=[[1, 128]],
            compare_op=ALU.is_ge, fill=0.0, base=0, channel_multiplier=-1,
        )
        tri = rounded("tri", tri_f)

        # dtri256[p, f] = 1 if f - 128 >= p else 0 (s=3 diag tile, 256 wide)
        dtri_f = cn.tile([128, 256], F32, tag="dtri_f")
        nc.vector.memset(dtri_f, 1.0)
        nc.gpsimd.affine_select(
            out=dtri_f, in_=dtri_f, pattern=[[1, 256]],
            compare_op=ALU.is_ge, fill=0.0, base=-128, channel_multiplier=-1,
        )
        dtri = rounded("dtri", dtri_f)

        ones16 = cn.tile([128, 16], F32, tag="ones16")
        nc.vector.memset(ones16, 1.0)

        # first x tile loads ahead of the weights so the first projection
        # matmuls are never waiting on the DMA queue
        xt00 = xload.tile([128, 8, TT], BF16, tag="xt")
        nc.sync.dma_start(out=xt00[:, 0:4, :], in_=xT3[:, 0:4, 0:TT])
        wq_t = wpool.tile([128, 8, MLOC], BF16, tag="wq")
        nc.sync.dma_start(out=wq_t, in_=wqT.ap().rearrange("(o p) m -> p o m", p=128))
        nc.sync.dma_start(out=xt00[:, 4:8, :], in_=xT3[:, 4:8, 0:TT])
        wk_t = wpool.tile([128, 8, MLOC], BF16, tag="wk")
        nc.sync.dma_start(out=wk_t, in_=wkT.ap().rearrange("(o p) m -> p o m", p=128))
        wv_t = wpool.tile([128, 8, MLOC], BF16, tag="wv")
        nc.sync.dma_start(out=wv_t, in_=wvT.ap().rearrange("(o p) m -> p o m", p=128))

        wo_t = wpool.tile([128, D], BF16, tag="wo")
        nc.sync.dma_start(out=wo_t, in_=woT.ap())

        # ---- residents ----
        qTn = [qk_res.tile([128, S], F32R, tag=f"qTn{b}", name=f"qTn{b}") for b in range(B)]
        kTn = [qk_res.tile([128, S], F32R, tag=f"kTn{b}", name=f"kTn{b}") for b in range(B)]
        # v_aug[b]: [128(t within j-block), jb, 130] = [v_h0 | 1 | v_h1 | 1]
        vaug = [qk_res.tile([128, NJB, 130], F32R, tag=f"vaug{b}", name=f"vaug{b}") for b in range(B)]
        for b in range(B):
            nc.vector.tensor_copy(vaug[b][:, :, 64], ones16)
            nc.vector.tensor_copy(vaug[b][:, :, 129], ones16)

        # ---- phase A: one 512-token tile of projections + norms ----
        # phase A is software-pipelined as front/back halves: the front ends
        # at the packed quake rsqrt (DVE); the back picks up with the unpack
        # transposes. Emitting front(t+1) between them gives PE independent
        # matmul work to chew while the DVE chain of tile t drains.
        def load_x(b, tt):
            t0g = b * S + tt * TT
            xt = xload.tile([128, 8, TT], BF16, tag="xt")
            nc.sync.dma_start(out=xt[:, 0:4, :], in_=xT3[:, 0:4, t0g : t0g + TT])
            nc.sync.dma_start(out=xt[:, 4:8, :], in_=xT3[:, 4:8, t0g : t0g + TT])
            return xt

        def phase_a_front(b, tt, xt=None):
            t0 = tt * TT
            if xt is None:
                xt = load_x(b, tt)
            # q/k projections first: their PSUM tiles drain quickly (DVE
            # staging copy + ACT square run in parallel straight off PSUM)
            sqs = {}
            for w_t, which in ((wq_t, "q"), (wk_t, "k")):
                dest = qTn[b] if which == "q" else kTn[b]
                sl = dest[:, t0 : t0 + TT]
                ps = mmp.tile([128, TT], F32, tag="mm")
                for k in range(8):
                    nc.tensor.matmul(
                        ps, w_t[:, k, :], xt[:, k, :],
                        start=(k == 0), stop=(k == 7)
                    )
                if b == 0:
                    nc.scalar.copy(sl, ps)
                else:
                    nc.vector.tensor_copy(sl, ps)
                sq = sqp.tile([128, TT], F32R, tag="sq")
                if b == 0:
                    nc.scalar.square(sq, ps)
                else:
                    # batch 1's squares run while ACT is busy with batch 0's
                    # attention exps; DVE has more slack in that window
                    nc.vector.tensor_mul(sq, sl, sl)
                del ps
                sqs[which] = sq
            # v: project straight into [token, dim] layout (lhsT = x chunk)
            vt = mmp.tile([128, TT], F32, tag="mm")
            for sub in range(4):
                c0 = 128 * sub
                for k in range(8):
                    nc.tensor.matmul(
                        vt[:, c0 : c0 + 128],
                        xt[:, k, c0 : c0 + 128],
                        wv_t[:, k, :],
                        start=(k == 0), stop=(k == 7)
                    )
            for sub in range(4):
                jb = tt * 4 + sub
                c0 = 128 * sub
                nc.vector.tensor_copy(
                    vaug[b][:, jb, 0:130].rearrange(
                        "p (two c) -> p two c", two=2)[:, :, 0:64],
                    vt[:, c0 : c0 + 128].rearrange(
                        "p (two c) -> p two c", two=2),
                )
            del vt, xt
            # stage per-head sumsq rows to SBUF (ACT Copy is in every act
            # table, so this staging costs no table switch); separate q/k
            # tiles keep every partition start at 0 (32-align rule)
            sts = {}
            for which in ("q", "k"):
                ss = mmp.tile([2, TT], F32, tag="mm")
                nc.tensor.matmul(ss, eye2, sqs[which], start=True, stop=True)
                stw = nwt.tile([2, TT], F32, tag="st" + which,
                               name=f"st{which}{b}_{tt}")
                nc.scalar.copy(stw, ss)
                sts[which] = stw
                del ss
            del sqs
            # packed quake rsqrt: PE-transpose the [2, 512] rows into
            # [128, 16] so the DVE Newton ops run on 16-wide rows instead of
            # 512-wide ones (transposes instead of DMAs keep the chain
            # latency off the PE critical path). ssP cols 4c+{0,1} = q heads,
            # 4c+{2,3} = k heads for token chunk c.
            ssP = mmp.tile([128, 16], F32, tag="mm")
            for c in range(4):
                nc.tensor.transpose(
                    ssP[:, 4 * c : 4 * c + 2],
                    sts["q"][:, 128 * c : 128 * c + 128],
                    ident4[0:2, 0:2],
                )
                nc.tensor.transpose(
                    ssP[:, 4 * c + 2 : 4 * c + 4],
                    sts["k"][:, 128 * c : 128 * c + 128],
                    ident4[0:2, 0:2],
                )
            del sts
            y = nwt.tile([128, 16], F32, tag="nwt_y", name=f"y{b}_{tt}")
            t1 = nwt.tile([128, 16], F32, tag="nwt_t", name=f"t{b}_{tt}")
            v0 = nwt.tile([128, 16], F32, tag="nwt_v", name=f"v{b}_{tt}")
            # seed + ssP staging on DVE (Pool cannot read PSUM), Newton on
            # the otherwise-idle Pool engine so the chain does not queue
            # behind DVE's staging copies
            nc.vector.tensor_copy(v0, ssP)
            nc.vector.tensor_scalar(
                y.bitcast(mybir.dt.int32), ssP.bitcast(mybir.dt.int32),
                1, None, ALU.logical_shift_right,
            )
            del ssP
            nc.gpsimd.tensor_scalar(
                y.bitcast(mybir.dt.int32), y.bitcast(mybir.dt.int32),
                -1, 0x5F3759DF, ALU.mult, ALU.add,
            )
            # 2 Newton iterations: rel err ~4e-6, far below the bf16 input
            # quantization already accepted
            for _ in range(2):
                nc.gpsimd.tensor_mul(t1, y, y)
                nc.gpsimd.tensor_mul(t1, t1, v0)
                nc.gpsimd.tensor_scalar(t1, t1, -0.5, 1.5, ALU.mult, ALU.add)
                nc.gpsimd.tensor_mul(y, y, t1)
            del t1, v0
            return y

        def phase_a_back(b, tt, y):
            t0 = tt * TT
            # unpack back to [4, 512] rows via transposes, one staging copy
            rtP = mmp.tile([4, TT], F32, tag="mm")
            for c in range(4):
                nc.tensor.transpose(
                    rtP[:, 128 * c : 128 * c + 128], y[:, 4 * c : 4 * c + 4],
                    ident,
                )
            del y
            rt = nwt.tile([4, TT], F32R, tag="rt", name=f"rt{b}_{tt}")
            nc.scalar.copy(rt, rtP)
            del rtP
            for which, eyeT8 in (("q", eyeT8q), ("k", eyeT8k)):
                dest = qTn[b] if which == "q" else kTn[b]
                sl = dest[:, t0 : t0 + TT]
                bc = mmp.tile([128, TT], F32, tag="mm")
                nc.tensor.matmul(bc, eyeT8, rt, start=True, stop=True)
                nc.vector.tensor_mul(sl, sl, bc)
                del bc
            del rt

        # ---- phase B/C: attention + output projection ----
        def qk(sc_slice, b, h, jbl, i0, iw):
            """scoresT[j, i] block: lhsT = kT [64, 128] (j), rhs = qT [64, iw]."""
            nc.tensor.matmul(
                sc_slice,
                kTn[b][64 * h : 64 * h + 64, 128 * jbl : 128 * jbl + 128],
                qTn[b][64 * h : 64 * h + 64, i0 : i0 + iw],
                start=True,
                stop=True,
            )

        def pv(yt, b, h, jbl, u_slice, icol, first, last):
            nc.tensor.matmul(
                yt[:, icol : icol + u_slice.shape[-1]],
                vaug[b][:, jbl, 65 * h : 65 * h + 65],
                u_slice,
                start=first,
                stop=last,
            )

        def attn_core(b, n):
            i0 = n * TT
            ytt = ytn.tile([128, TT], BF16, tag="ytt")
            for h in range(HLOC):
                yt = ytp.tile([65, TT], F32, tag="yt")
                # diagonal first: 4 j-blocks, trapezoid widths + triangular
                # masks. Leading with them keeps the Pool mask latency off
                # the tail of the PV accumulation chain.
                # u1 cols: [0:512]@i0 (jb0), [512:896]@i0+128 (jb0+1)
                # u2 cols: [0:256]@i0+256 (jb0+2), [256:512]@i0+256 (jb0+3)
                jb0 = 4 * n
                sc = scp.tile([128, 1024], F32, tag="sc")
                qk(sc[:, 0:512], b, h, jb0, i0, 512)
                qk(sc[:, 512:896], b, h, jb0 + 1, i0 + 128, 384)
                u1 = up.tile([128, 1024], F32R, tag="u")
                nc.scalar.activation(u1[:, 0:896], sc[:, 0:896], AF.Exp,
                                     scale=ISQDK)
                del sc
                sc = scp.tile([128, 1024], F32, tag="sc")
                qk(sc[:, 0:256], b, h, jb0 + 2, i0 + 256, 256)
                qk(sc[:, 256:512], b, h, jb0 + 3, i0 + 256, 256)
                u2 = up.tile([128, 1024], F32R, tag="u")
                nc.scalar.activation(u2[:, 0:512], sc[:, 0:512], AF.Exp,
                                     scale=ISQDK)
                del sc
                nc.gpsimd.tensor_mul(u1[:, 0:128], u1[:, 0:128], tri)
                nc.gpsimd.tensor_mul(u1[:, 512:640], u1[:, 512:640], tri)
                nc.gpsimd.tensor_mul(u2[:, 0:128], u2[:, 0:128], tri)
                nc.gpsimd.tensor_mul(u2[:, 256:512], u2[:, 256:512], dtri)
                pv(yt, b, h, jb0, u1[:, 0:512], 0, first=True, last=False)
                pv(yt, b, h, jb0 + 1, u1[:, 512:896], 128, first=False,
                   last=False)
                del u1
                pv(yt, b, h, jb0 + 2, u2[:, 0:256], 256, first=False,
                   last=False)
                pv(yt, b, h, jb0 + 3, u2[:, 256:512], 256, first=False,
                   last=(n == 0))
                del u2
                # full j-blocks below the diagonal, two per scores tile
                for jp in range(2 * n):
                    jbl = 2 * jp
                    sc = scp.tile([128, 1024], F32, tag="sc")
                    qk(sc[:, 0:512], b, h, jbl, i0, 512)
                    qk(sc[:, 512:1024], b, h, jbl + 1, i0, 512)
                    u = up.tile([128, 1024], F32R, tag="u")
                    nc.scalar.activation(u, sc, AF.Exp, scale=ISQDK)
                    del sc
                    pv(yt, b, h, jbl, u[:, 0:512], 0, first=False, last=False)
                    pv(yt, b, h, jbl + 1, u[:, 512:1024], 0, first=False,
                       last=(jp == 2 * n - 1))
                    del u

                # normalize this head: rden = 1/denominator, broadcast via
                # K=1 matmul, stage yt to SBUF (one-PSUM-input rule), multiply
                rden = rsp.tile([1, TT], F32R, tag="rden")
                with nc.allow_low_precision(reason="fp32r matmul operand"):
                    nc.vector.reciprocal(rden, yt[64:65, :])
                bc2 = mmp.tile([64, TT], F32, tag="mm")
                nc.tensor.matmul(bc2, ones1, rden, start=True, stop=True)
                del rden
                ytsb = ytn.tile([64, TT], F32, tag="ytsb")
                nc.vector.tensor_copy(ytsb, yt[0:64, :])
                del yt
                if h == 0:
                    nc.vector.tensor_mul(ytt[0:64, :], ytsb, bc2)
                else:
                    y1 = ytn.tile([64, TT], BF16, tag="y1")
                    nc.vector.tensor_mul(y1, ytsb, bc2)
                    # partition shift 0..63 -> 64..127 via SBUF-to-SBUF DMA
                    nc.sync.dma_start(out=ytt[64:128, :], in_=y1)
                    del y1
                del ytsb, bc2
            return ytt

        def attn_out(b, n, ytt, drain=False):
            # phase C: out[t, :] = ytt.T @ woT, 128-token sub-blocks, staged
            # to a bf16 tile; per-ts DMAs start draining as soon as each
            # 128-token row block is converted. drain=True alternates the
            # staging copies across DVE/ACT so the kernel tail is not paced
            # by a single engine.
            os = osp.tile([128, 4, D], BF16, tag="os")
            blk0 = b * (S // 128) + 4 * n
            for ts in range(4):
                for nn in range(2):
                    op = mmp.tile([128, 512], F32, tag="mm")
                    nc.tensor.matmul(
                        op,
                        ytt[:, 128 * ts : 128 * ts + 128],
                        wo_t[:, 512 * nn : 512 * nn + 512],
                        start=True,
                        stop=True,
                    )
                    dst = os[:, ts, 512 * nn : 512 * nn + 512]
                    on_act = (ts == 3) if not drain else ((2 * ts + nn) % 2 == 1)
                    if on_act:
                        nc.scalar.copy(dst, op)
                    else:
                        nc.vector.tensor_copy(dst, op)
                    del op
                nc.sync.dma_start(
                    out=out3[:, blk0 + ts : blk0 + ts + 1, :],
                    in_=os[:, ts : ts + 1, :],
                )
            del ytt, os

        # ---- emission: software-pipelined. Phase-A fronts run one tile
        # ahead of backs; attention cores run one block ahead of their
        # output projections; phase A(b1) threads between attention(b0)
        # blocks so PE always has independent matmul work queued. ----
        ys = {}
        ys[(0, 0)] = phase_a_front(0, 0, xt=xt00)
        for tt in range(1, NTB):
            ys[(0, tt)] = phase_a_front(0, tt)
            phase_a_back(0, tt - 1, ys.pop((0, tt - 1)))
        phase_a_back(0, NTB - 1, ys.pop((0, NTB - 1)))

        ytt_prev = None  # (b, n, ytt) awaiting its output projection
        border = (3, 2, 1, 0)
        for i, n in enumerate(range(NTB)):
            ytt = attn_core(0, n)
            if ytt_prev is not None:
                attn_out(*ytt_prev)
            ytt_prev = (0, n, ytt)
            ys[(1, i)] = phase_a_front(1, i)
            if i > 0:
                phase_a_back(1, i - 1, ys.pop((1, i - 1)))
        phase_a_back(1, NTB - 1, ys.pop((1, NTB - 1)))
        for n in range(NTB):
            ytt = attn_core(1, n)
            attn_out(*ytt_prev, drain=(n == NTB - 1))
            ytt_prev = (1, n, ytt)
        attn_out(*ytt_prev, drain=True)


_NC_CACHE = None


def _get_nc():
    global _NC_CACHE
    if _NC_CACHE is None:
        _NC_CACHE = build_kernel()
    return _NC_CACHE


def make_in_maps(x, w_q, w_k, w_v, w_o):
    import ml_dtypes

    bf16 = ml_dtypes.bfloat16
    x = np.ascontiguousarray(np.asarray(x, dtype=np.float32))
    w_q = np.asarray(w_q, dtype=np.float32)
    w_k = np.asarray(w_k, dtype=np.float32)
    w_v = np.asarray(w_v, dtype=np.float32)
    w_o = np.asarray(w_o, dtype=np.float32)

    xT = np.ascontiguousarray(x.reshape(T, D).T).astype(bf16)  # [D, T]
    in_maps = []
    for c in range(NCORES):
        hs = slice(c * MLOC, (c + 1) * MLOC)
        in_maps.append(
            {
                "xT": xT,
                "wqT": np.ascontiguousarray(w_q[hs, :].T).astype(bf16),
                "wkT": np.ascontiguousarray(w_k[hs, :].T).astype(bf16),
                "wvT": np.ascontiguousarray(w_v[hs, :].T).astype(bf16),
                "woT": np.ascontiguousarray(w_o[:, hs].T).astype(bf16),
            }
        )
    return in_maps


def combine_outputs(results):
    acc = results[0]["out"].astype(np.float64)
    for c in range(1, NCORES):
        acc += results[c]["out"].astype(np.float64)
    return acc.astype(np.float32).reshape(B, S, D)


def kernel(x, w_q, w_k, w_v, w_o):
    in_maps = make_in_maps(x, w_q, w_k, w_v, w_o)
    nc = _get_nc()
    res = run_bass_kernel_spmd(nc, in_maps, core_ids=list(range(NCORES)))
    return combine_outputs(res.results)


if __name__ == "__main__":
    rng = np.random.default_rng(0)
    ins = {
        "x": rng.standard_normal((B, S, D), dtype=np.float32),
        "w_q": rng.standard_normal((D, D), dtype=np.float32) * 0.02,
        "w_k": rng.standard_normal((D, D), dtype=np.float32) * 0.02,
        "w_v": rng.standard_normal((D, D), dtype=np.float32) * 0.02,
        "w_o": rng.standard_normal((D, D), dtype=np.float32) * 0.02,
    }
    y = kernel(**ins)
    print("kernel output", y.shape, y.dtype, float(np.abs(y).max()))


# revision 59
# speedup vs baseline: 1.0176x; 1.0176x over previous
"""Causal multi-head self-attention (QK-RMSNorm + tanh softcap) on 8 trn2 cores.

Problem (hardcoded): x [2, 2048, 1024], w_q/w_k/w_v/w_o [1024, 1024] fp32,
H=16 heads, dk=64, softcap 50, causal, out = softmax-attn @ w_o.T.

Sharding: head-parallel. Core c owns heads {2c, 2c+1} (128 local dims):
  - w_q/w_k/w_v sliced by rows -> per-core [128, 1024]; host pre-transposes.
  - w_o sliced by columns -> per-core [1024, 128]; host pre-transposes.
  - x is replicated (host pre-transposed to xT [1024, 4096], bf16).
  - Each core emits a full-shape bf16 partial output [4096, 1024]; host sums.

Numerics: the tanh softcap is dropped (|logits| <= 8 by Cauchy-Schwarz after
QK RMS norm, so tanh(s/50)*50 ~ s to ~2e-3 relative; measured end-to-end
error vs the fp32 reference is ~3.5e-3 against a 2e-2 gate, including the
bf16 input/output quantization). exp needs no running max (logits bounded).

On-core pipeline per 512-token tile (matmuls keyed on the bf16/f32r moving
operand run at 1 cycle/row):
  A) q/k: ps = wT.T @ x (8 k-chunk matmuls, bf16 in, f32 PSUM), staged to
     resident f32r qTn/kTn; per-head sumsq via eye2 matmul of sq = ACT
     Square(ps); rstd = 1/sqrt(ss) via a packed quake-Newton rsqrt
     (PE-transpose [2,512] rows into [128,16], seed on DVE, Newton iters on
     the otherwise-idle Pool engine, transpose back); the x8 RMS factor is
     folded into the eyeT8q/k broadcast matmuls; in-place normalize on DVE.
     v: projected directly into [token, dim] layout by 32 small matmuls
     (lhsT = x chunk), staged into vaug with ones columns for the softmax
     denominator. Fronts (through quake) and backs (unpack + normalize)
     are software-pipelined one tile apart.
  B) per (batch, head, 512-query block): scoresT[j, i] = k.T @ q blocks into
     [128, 1024] PSUM, exp straight off PSUM (scale 1/8) into f32r u tiles
     (diagonal trapezoid first so Pool mask latency stays off the PV tail),
     causal via block skip + triangular mask multiply (Pool), PV accumulate
     with v stationary (denominator rides along as row 64), normalize via
     reciprocal + K=1 ones matmul broadcast + DVE multiply. Output
     projections lag their attention core by one block.
  C) out[t, :] = ytt.T @ w_oT per 128-token block; PSUM staged to a bf16
     [128, 4, 1024] tile (DVE, with ACT taking a share; alternating on the
     drain blocks) and DMAd per 128-token row block.
"""

import sys

for _p in ("/opt/trn_rl_repo",):
    if _p not in sys.path:
        sys.path.insert(0, _p)

import numpy as np

import concourse.bacc as bacc
import concourse.tile as tile
from concourse import mybir
from concourse.bass_utils import run_bass_kernel_spmd

F32 = mybir.dt.float32
F32R = mybir.dt.float32r
BF16 = mybir.dt.bfloat16
AF = mybir.ActivationFunctionType
ALU = mybir.AluOpType

B, S, D = 2, 2048, 1024
H, DK = 16, 64
NCORES = 8
HLOC = H // NCORES          # 2 heads per core
MLOC = HLOC * DK            # 128 local head dims
T = B * S                   # 4096 tokens

TT = 512                    # token tile (phase A, also query i-block)
NTB = S // TT               # 4 token tiles per batch
JB = 128                    # key j-block
NJB = S // JB               # 16 j-blocks per batch
ISQDK = 1.0 / 8.0           # 1/sqrt(64)


def build_kernel():
    nc = bacc.Bacc("TRN2", target_bir_lowering=False, debug=False)

    xT = nc.dram_tensor("xT", [D, T], BF16, kind="ExternalInput")
    wqT = nc.dram_tensor("wqT", [D, MLOC], BF16, kind="ExternalInput")
    wkT = nc.dram_tensor("wkT", [D, MLOC], BF16, kind="ExternalInput")
    wvT = nc.dram_tensor("wvT", [D, MLOC], BF16, kind="ExternalInput")
    woT = nc.dram_tensor("woT", [MLOC, D], BF16, kind="ExternalInput")
    out = nc.dram_tensor("out", [T, D], BF16, kind="ExternalOutput")

    xT3 = xT.ap().rearrange("(o p) t -> p o t", p=128)      # [128, 8, 4096]
    out3 = out.ap().rearrange("(blk p) d -> p blk d", p=128)  # [128, 32, 1024]

    with tile.TileContext(nc) as tc:
        _emit(nc, tc, xT3, wqT, wkT, wvT, woT, out3)

    nc.compile()
    return nc


def _emit(nc, tc, xT3, wqT, wkT, wvT, woT, out3):
    from contextlib import ExitStack

    ctx = ExitStack()
    with ctx:
        cn = ctx.enter_context(tc.tile_pool(name="cn", bufs=1))
        # small staging tiles for the packed quake rsqrt chain
        nwt = ctx.enter_context(tc.tile_pool(name="nwt", bufs=8))
        xload = ctx.enter_context(tc.tile_pool(name="xload", bufs=3))
        wpool = ctx.enter_context(tc.tile_pool(name="wpool", bufs=1))
        qk_res = ctx.enter_context(tc.tile_pool(name="qk_res", bufs=1))
        sqp = ctx.enter_context(tc.tile_pool(name="sqp", bufs=3))
        rsp = ctx.enter_context(tc.tile_pool(name="rsp", bufs=4))
        up = ctx.enter_context(tc.tile_pool(name="up", bufs=6))
        ytn = ctx.enter_context(tc.tile_pool(name="ytn", bufs=3))
        osp = ctx.enter_context(tc.tile_pool(name="osp", bufs=2))
        # PSUM: sc 2x[128,1024] (4 banks) + yt 2x[65,512] (2) + mm 2x[128,512]
        # (2) = 8 banks. The whole rstd chain runs through PE transposes and
        # engine ops (no DMA hops), so its mm-slot reuse drains fast enough
        # to share the single mm tag with proj/outproj tiles.
        scp = ctx.enter_context(tc.tile_pool(name="scp", bufs=2, space="PSUM"))
        ytp = ctx.enter_context(tc.tile_pool(name="ytp", bufs=2, space="PSUM"))
        mmp = ctx.enter_context(tc.tile_pool(name="mmp", bufs=2, space="PSUM"))

        # ---- constants ----
        def rounded(name, f32_tile):
            t = cn.tile(list(f32_tile.shape), F32R, tag=name + "_r", name=name + "_r")
            nc.vector.tensor_copy(t, f32_tile)
            return t

        eye2_f = cn.tile([128, 2], F32, tag="eye2_f")
        nc.vector.memset(eye2_f, 0.0)
        nc.vector.memset(eye2_f[0:64, 0:1], 1.0)
        nc.vector.memset(eye2_f[64:128, 1:2], 1.0)
        eye2 = rounded("eye2", eye2_f)

        ident = cn.tile([128, 128], F32, tag="ident")
        nc.vector.memset(ident, 1.0)
        nc.gpsimd.affine_select(
            out=ident, in_=ident, pattern=[[1, 128]],
            compare_op=ALU.is_equal, fill=0.0, base=0, channel_multiplier=-1,
        )

        ident4 = cn.tile([4, 4], F32, tag="ident4")
        nc.vector.memset(ident4, 1.0)
        nc.gpsimd.affine_select(
            out=ident4, in_=ident4, pattern=[[1, 4]],
            compare_op=ALU.is_equal, fill=0.0, base=0, channel_multiplier=-1,
        )

        # eyeT8q[p, f] = 8 iff 0 <= f - 64p < 64 (rows 2,3 fall out as zero):
        # broadcasts rt rows 0:2 (q rstd per head) across their 64 partitions,
        # folding in the x8 RMS factor (rstd = 1/sqrt(sumsq); rms norm needs
        # 8/sqrt(sumsq)). eyeT8k does the same for rt rows 2:4 (k rstd).
        eyeT8q_f = cn.tile([4, 128], F32, tag="eyeT8q_f")
        nc.vector.memset(eyeT8q_f, 8.0)
        nc.gpsimd.affine_select(
            out=eyeT8q_f, in_=eyeT8q_f, pattern=[[1, 128]],
            compare_op=ALU.is_ge, fill=0.0, base=0, channel_multiplier=-64,
        )
        nc.gpsimd.affine_select(
            out=eyeT8q_f, in_=eyeT8q_f, pattern=[[-1, 128]],
            compare_op=ALU.is_ge, fill=0.0, base=63, channel_multiplier=64,
        )
        eyeT8q = rounded("eyeT8q", eyeT8q_f)

        eyeT8k_f = cn.tile([4, 128], F32, tag="eyeT8k_f")
        nc.vector.memset(eyeT8k_f, 8.0)
        nc.gpsimd.affine_select(
            out=eyeT8k_f, in_=eyeT8k_f, pattern=[[1, 128]],
            compare_op=ALU.is_ge, fill=0.0, base=128, channel_multiplier=-64,
        )
        nc.gpsimd.affine_select(
            out=eyeT8k_f, in_=eyeT8k_f, pattern=[[-1, 128]],
            compare_op=ALU.is_ge, fill=0.0, base=-65, channel_multiplier=64,
        )
        eyeT8k = rounded("eyeT8k", eyeT8k_f)

        ones1_f = cn.tile([1, 64], F32, tag="ones1_f")
        nc.vector.memset(ones1_f, 1.0)
        ones1 = rounded("ones1", ones1_f)

        # tri128[p, f] = 1 if f >= p else 0  (keep i>=j in [j, i] tiles)
        tri_f = cn.tile([128, 128], F32, tag="tri_f")
        nc.vector.memset(tri_f, 1.0)
        nc.gpsimd.affine_select(
            out=tri_f, in_=tri_f, pattern=[[1, 128]],
            compare_op=ALU.is_ge, fill=0.0, base=0, channel_multiplier=-1,
        )
        tri = rounded("tri", tri_f)

        # dtri256[p, f] = 1 if f - 128 >= p else 0 (s=3 diag tile, 256 wide)
        dtri_f = cn.tile([128, 256], F32, tag="dtri_f")
        nc.vector.memset(dtri_f, 1.0)
        nc.gpsimd.affine_select(
            out=dtri_f, in_=dtri_f, pattern=[[1, 256]],
            compare_op=ALU.is_ge, fill=0.0, base=-128, channel_multiplier=-1,
        )
        dtri = rounded("dtri", dtri_f)

        ones16 = cn.tile([128, 16], F32, tag="ones16")
        nc.vector.memset(ones16, 1.0)

        # first x tile loads ahead of the weights so the first projection
        # matmuls are never waiting on the DMA queue
        xt00 = xload.tile([128, 8, TT], BF16, tag="xt")
        nc.sync.dma_start(out=xt00[:, 0:4, :], in_=xT3[:, 0:4, 0:TT])
        wq_t = wpool.tile([128, 8, MLOC], BF16, tag="wq")
        nc.sync.dma_start(out=wq_t, in_=wqT.ap().rearrange("(o p) m -> p o m", p=128))
        nc.sync.dma_start(out=xt00[:, 4:8, :], in_=xT3[:, 4:8, 0:TT])
        wk_t = wpool.tile([128, 8, MLOC], BF16, tag="wk")
        nc.sync.dma_start(out=wk_t, in_=wkT.ap().rearrange("(o p) m -> p o m", p=128))
        wv_t = wpool.tile([128, 8, MLOC], BF16, tag="wv")
        nc.sync.dma_start(out=wv_t, in_=wvT.ap().rearrange("(o p) m -> p o m", p=128))

        wo_t = wpool.tile([128, D], BF16, tag="wo")
        nc.sync.dma_start(out=wo_t, in_=woT.ap())

        # ---- residents ----
        qTn = [qk_res.tile([128, S], F32R, tag=f"qTn{b}", name=f"qTn{b}") for b in range(B)]
        kTn = [qk_res.tile([128, S], F32R, tag=f"kTn{b}", name=f"kTn{b}") for b in range(B)]
        # v_aug[b]: [128(t within j-block), jb, 130] = [v_h0 | 1 | v_h1 | 1]
        vaug = [qk_res.tile([128, NJB, 130], F32R, tag=f"vaug{b}", name=f"vaug{b}") for b in range(B)]
        for b in range(B):
            nc.vector.tensor_copy(vaug[b][:, :, 64], ones16)
            nc.vector.tensor_copy(vaug[b][:, :, 129], ones16)

        # ---- phase A: one 512-token tile of projections + norms ----
        # phase A is software-pipelined as front/back halves: the front ends
        # at the packed quake rsqrt (DVE); the back picks up with the unpack
        # transposes. Emitting front(t+1) between them gives PE independent
        # matmul work to chew while the DVE chain of tile t drains.
        def load_x(b, tt):
            t0g = b * S + tt * TT
            xt = xload.tile([128, 8, TT], BF16, tag="xt")
            nc.sync.dma_start(out=xt[:, 0:4, :], in_=xT3[:, 0:4, t0g : t0g + TT])
            nc.sync.dma_start(out=xt[:, 4:8, :], in_=xT3[:, 4:8, t0g : t0g + TT])
            return xt

        def phase_a_front(b, tt, xt=None):
            t0 = tt * TT
            if xt is None:
                xt = load_x(b, tt)
            # q/k projections first: their PSUM tiles drain quickly (DVE
            # staging copy + ACT square run in parallel straight off PSUM)
            sqs = {}
            for w_t, which in ((wq_t, "q"), (wk_t, "k")):
                dest = qTn[b] if which == "q" else kTn[b]
                sl = dest[:, t0 : t0 + TT]
                ps = mmp.tile([128, TT], F32, tag="mm")
                for k in range(8):
                    nc.tensor.matmul(
                        ps, w_t[:, k, :], xt[:, k, :],
                        start=(k == 0), stop=(k == 7)
                    )
                if b == 0:
                    nc.scalar.copy(sl, ps)
                else:
                    nc.vector.tensor_copy(sl, ps)
                sq = sqp.tile([128, TT], F32R, tag="sq")
                if b == 0:
                    nc.scalar.square(sq, ps)
                else:
                    # batch 1's squares run while ACT is busy with batch 0's
                    # attention exps; DVE has more slack in that window
                    nc.vector.tensor_mul(sq, sl, sl)
                del ps
                sqs[which] = sq
            # v: project straight into [token, dim] layout (lhsT = x chunk)
            vt = mmp.tile([128, TT], F32, tag="mm")
            for sub in range(4):
                c0 = 128 * sub
                for k in range(8):
                    nc.tensor.matmul(
                        vt[:, c0 : c0 + 128],
                        xt[:, k, c0 : c0 + 128],
                        wv_t[:, k, :],
                        start=(k == 0), stop=(k == 7)
                    )
            for sub in range(4):
                jb = tt * 4 + sub
                c0 = 128 * sub
                nc.vector.tensor_copy(
                    vaug[b][:, jb, 0:130].rearrange(
                        "p (two c) -> p two c", two=2)[:, :, 0:64],
                    vt[:, c0 : c0 + 128].rearrange(
                        "p (two c) -> p two c", two=2),
                )
            del vt, xt
            # stage per-head sumsq rows to SBUF (ACT Copy is in every act
            # table, so this staging costs no table switch); separate q/k
            # tiles keep every partition start at 0 (32-align rule)
            sts = {}
            for which in ("q", "k"):
                ss = mmp.tile([2, TT], F32, tag="mm")
                nc.tensor.matmul(ss, eye2, sqs[which], start=True, stop=True)
                stw = nwt.tile([2, TT], F32, tag="st" + which,
                               name=f"st{which}{b}_{tt}")
                nc.scalar.copy(stw, ss)
                sts[which] = stw
                del ss
            del sqs
            # packed quake rsqrt: PE-transpose the [2, 512] rows into
            # [128, 16] so the DVE Newton ops run on 16-wide rows instead of
            # 512-wide ones (transposes instead of DMAs keep the chain
            # latency off the PE critical path). ssP cols 4c+{0,1} = q heads,
            # 4c+{2,3} = k heads for token chunk c.
            ssP = mmp.tile([128, 16], F32, tag="mm")
            for c in range(4):
                nc.tensor.transpose(
                    ssP[:, 4 * c : 4 * c + 2],
                    sts["q"][:, 128 * c : 128 * c + 128],
                    ident4[0:2, 0:2],
                )
                nc.tensor.transpose(
                    ssP[:, 4 * c + 2 : 4 * c + 4],
                    sts["k"][:, 128 * c : 128 * c + 128],
                    ident4[0:2, 0:2],
                )
            del sts
            y = nwt.tile([128, 16], F32, tag="nwt_y", name=f"y{b}_{tt}")
            t1 = nwt.tile([128, 16], F32, tag="nwt_t", name=f"t{b}_{tt}")
            v0 = nwt.tile([128, 16], F32, tag="nwt_v", name=f"v{b}_{tt}")
            # seed + ssP staging on DVE (Pool cannot read PSUM), Newton on
            # the otherwise-idle Pool engine so the chain does not queue
            # behind DVE's staging copies
            nc.vector.tensor_copy(v0, ssP)
            nc.vector.tensor_scalar(
                y.bitcast(mybir.dt.int32), ssP.bitcast(mybir.dt.int32),
                1, None, ALU.logical_shift_right,
            )
            del ssP
            nc.gpsimd.tensor_scalar(
                y.bitcast(mybir.dt.int32), y.bitcast(mybir.dt.int32),
                -1, 0x5F3759DF, ALU.mult, ALU.add,
            )
            # 2 Newton iterations: rel err ~4e-6, far below the bf16 input
            # quantization already accepted
            for _ in range(2):
                nc.gpsimd.tensor_mul(t1, y, y)
                nc.gpsimd.tensor_mul(t1, t1, v0)
                nc.gpsimd.tensor_scalar(t1, t1, -0.5, 1.5, ALU.mult, ALU.add)
                nc.gpsimd.tensor_mul(y, y, t1)
            del t1, v0
            return y

        def phase_a_back(b, tt, y):
            t0 = tt * TT
            # unpack back to [4, 512] rows via transposes, one staging copy
            rtP = mmp.tile([4, TT], F32, tag="mm")
            for c in range(4):
                nc.tensor.transpose(
                    rtP[:, 128 * c : 128 * c + 128], y[:, 4 * c : 4 * c + 4],
                    ident,
                )
            del y
            rt = nwt.tile([4, TT], F32R, tag="rt", name=f"rt{b}_{tt}")
            nc.scalar.copy(rt, rtP)
            del rtP
            for which, eyeT8 in (("q", eyeT8q), ("k", eyeT8k)):
                dest = qTn[b] if which == "q" else kTn[b]
                sl = dest[:, t0 : t0 + TT]
                bc = mmp.tile([128, TT], F32, tag="mm")
                nc.tensor.matmul(bc, eyeT8, rt, start=True, stop=True)
                nc.vector.tensor_mul(sl, sl, bc)
                del bc
            del rt

        # ---- phase B/C: attention + output projection ----
        def qk(sc_slice, b, h, jbl, i0, iw):
            """scoresT[j, i] block: lhsT = kT [64, 128] (j), rhs = qT [64, iw]."""
            nc.tensor.matmul(
                sc_slice,
                kTn[b][64 * h : 64 * h + 64, 128 * jbl : 128 * jbl + 128],
                qTn[b][64 * h : 64 * h + 64, i0 : i0 + iw],
                start=True,
                stop=True,
            )

        def pv(yt, b, h, jbl, u_slice, icol, first, last):
            nc.tensor.matmul(
                yt[:, icol : icol + u_slice.shape[-1]],
                vaug[b][:, jbl, 65 * h : 65 * h + 65],
                u_slice,
                start=first,
                stop=last,
            )

        def attn_core(b, n):
            i0 = n * TT
            ytt = ytn.tile([128, TT], BF16, tag="ytt")
            for h in range(HLOC):
                yt = ytp.tile([65, TT], F32, tag="yt")
                # diagonal first: 4 j-blocks, trapezoid widths + triangular
                # masks. Leading with them keeps the Pool mask latency off
                # the tail of the PV accumulation chain.
                # u1 cols: [0:512]@i0 (jb0), [512:896]@i0+128 (jb0+1)
                # u2 cols: [0:256]@i0+256 (jb0+2), [256:512]@i0+256 (jb0+3)
                jb0 = 4 * n
                sc = scp.tile([128, 1024], F32, tag="sc")
                qk(sc[:, 0:512], b, h, jb0, i0, 512)
                qk(sc[:, 512:896], b, h, jb0 + 1, i0 + 128, 384)
                u1 = up.tile([128, 1024], F32R, tag="u")
                nc.scalar.activation(u1[:, 0:896], sc[:, 0:896], AF.Exp,
                                     scale=ISQDK)
                del sc
                sc = scp.tile([128, 1024], F32, tag="sc")
                qk(sc[:, 0:256], b, h, jb0 + 2, i0 + 256, 256)
                qk(sc[:, 256:512], b, h, jb0 + 3, i0 + 256, 256)
                u2 = up.tile([128, 1024], F32R, tag="u")
                nc.scalar.activation(u2[:, 0:512], sc[:, 0:512], AF.Exp,
                                     scale=ISQDK)
                del sc
                nc.gpsimd.tensor_mul(u1[:, 0:128], u1[:, 0:128], tri)
                nc.gpsimd.tensor_mul(u1[:, 512:640], u1[:, 512:640], tri)
                nc.gpsimd.tensor_mul(u2[:, 0:128], u2[:, 0:128], tri)
                nc.gpsimd.tensor_mul(u2[:, 256:512], u2[:, 256:512], dtri)
                # full j-blocks below the diagonal, two per scores tile; the
                # diag PVs run LAST so the Pool mask latency is hidden under
                # the off-diag stream
                for jp in range(2 * n):
                    jbl = 2 * jp
                    sc = scp.tile([128, 1024], F32, tag="sc")
                    qk(sc[:, 0:512], b, h, jbl, i0, 512)
                    qk(sc[:, 512:1024], b, h, jbl + 1, i0, 512)
                    u = up.tile([128, 1024], F32R, tag="u")
                    nc.scalar.activation(u, sc, AF.Exp, scale=ISQDK)
                    del sc
                    pv(yt, b, h, jbl, u[:, 0:512], 0, first=(jp == 0),
                       last=False)
                    pv(yt, b, h, jbl + 1, u[:, 512:1024], 0, first=False,
                       last=False)
                    del u
                pv(yt, b, h, jb0, u1[:, 0:512], 0, first=(n == 0), last=False)
                pv(yt, b, h, jb0 + 1, u1[:, 512:896], 128, first=False,
                   last=False)
                del u1
                pv(yt, b, h, jb0 + 2, u2[:, 0:256], 256, first=False,
                   last=False)
                pv(yt, b, h, jb0 + 3, u2[:, 256:512], 256, first=False,
                   last=True)
                del u2

                # normalize this head: rden = 1/denominator, broadcast via
                # K=1 matmul, stage yt to SBUF (one-PSUM-input rule), multiply
                rden = rsp.tile([1, TT], F32R, tag="rden")
                with nc.allow_low_precision(reason="fp32r matmul operand"):
                    nc.vector.reciprocal(rden, yt[64:65, :])
                bc2 = mmp.tile([64, TT], F32, tag="mm")
                nc.tensor.matmul(bc2, ones1, rden, start=True, stop=True)
                del rden
                ytsb = ytn.tile([64, TT], F32, tag="ytsb")
                nc.vector.tensor_copy(ytsb, yt[0:64, :])
                del yt
                if h == 0:
                    nc.vector.tensor_mul(ytt[0:64, :], ytsb, bc2)
                else:
                    y1 = ytn.tile([64, TT], BF16, tag="y1")
                    nc.vector.tensor_mul(y1, ytsb, bc2)
                    # partition shift 0..63 -> 64..127 via SBUF-to-SBUF DMA
                    nc.sync.dma_start(out=ytt[64:128, :], in_=y1)
                    del y1
                del ytsb, bc2
            return ytt

        def attn_out(b, n, ytt, drain=False):
            # phase C: out[t, :] = ytt.T @ woT, 128-token sub-blocks, staged
            # to a bf16 tile; per-ts DMAs start draining as soon as each
            # 128-token row block is converted. drain=True alternates the
            # staging copies across DVE/ACT so the kernel tail is not paced
            # by a single engine.
            os = osp.tile([128, 4, D], BF16, tag="os")
            blk0 = b * (S // 128) + 4 * n
            for ts in range(4):
                for nn in range(2):
                    op = mmp.tile([128, 512], F32, tag="mm")
                    nc.tensor.matmul(
                        op,
                        ytt[:, 128 * ts : 128 * ts + 128],
                        wo_t[:, 512 * nn : 512 * nn + 512],
                        start=True,
                        stop=True,
                    )
                    dst = os[:, ts, 512 * nn : 512 * nn + 512]
                    on_act = (ts == 3) if not drain else ((2 * ts + nn) % 2 == 1)
                    if on_act:
                        nc.scalar.copy(dst, op)
                    else:
                        nc.vector.tensor_copy(dst, op)
                    del op
                nc.sync.dma_start(
                    out=out3[:, blk0 + ts : blk0 + ts + 1, :],
                    in_=os[:, ts : ts + 1, :],
                )
            del ytt, os

        # ---- emission: software-pipelined. Phase-A fronts run one tile
        # ahead of backs; attention cores run one block ahead of their
        # output projections; phase A(b1) threads between attention(b0)
        # blocks so PE always has independent matmul work queued. ----
        ys = {}
        ys[(0, 0)] = phase_a_front(0, 0, xt=xt00)
        for tt in range(1, NTB):
            ys[(0, tt)] = phase_a_front(0, tt)
            phase_a_back(0, tt - 1, ys.pop((0, tt - 1)))
        phase_a_back(0, NTB - 1, ys.pop((0, NTB - 1)))

        ytt_prev = None  # (b, n, ytt) awaiting its output projection
        border = (3, 2, 1, 0)
        for i, n in enumerate(range(NTB)):
            ytt = attn_core(0, n)
            if ytt_prev is not None:
                attn_out(*ytt_prev)
            ytt_prev = (0, n, ytt)
            ys[(1, i)] = phase_a_front(1, i)
            if i > 0:
                phase_a_back(1, i - 1, ys.pop((1, i - 1)))
        phase_a_back(1, NTB - 1, ys.pop((1, NTB - 1)))
        for n in range(NTB):
            ytt = attn_core(1, n)
            attn_out(*ytt_prev, drain=(n == NTB - 1))
            ytt_prev = (1, n, ytt)
        attn_out(*ytt_prev, drain=True)


_NC_CACHE = None


def _get_nc():
    global _NC_CACHE
    if _NC_CACHE is None:
        _NC_CACHE = build_kernel()
    return _NC_CACHE


def make_in_maps(x, w_q, w_k, w_v, w_o):
    import ml_dtypes

    bf16 = ml_dtypes.bfloat16
    x = np.ascontiguousarray(np.asarray(x, dtype=np.float32))
    w_q = np.asarray(w_q, dtype=np.float32)
    w_k = np.asarray(w_k, dtype=np.float32)
    w_v = np.asarray(w_v, dtype=np.float32)
    w_o = np.asarray(w_o, dtype=np.float32)

    xT = np.ascontiguousarray(x.reshape(T, D).T).astype(bf16)  # [D, T]
    in_maps = []
    for c in range(NCORES):
        hs = slice(c * MLOC, (c + 1) * MLOC)
        in_maps.append(
            {
                "xT": xT,
                "wqT": np.ascontiguousarray(w_q[hs, :].T).astype(bf16),
                "wkT": np.ascontiguousarray(w_k[hs, :].T).astype(bf16),
                "wvT": np.ascontiguousarray(w_v[hs, :].T).astype(bf16),
                "woT": np.ascontiguousarray(w_o[:, hs].T).astype(bf16),
            }
        )
    return in_maps


def combine_outputs(results):
    acc = results[0]["out"].astype(np.float64)
    for c in range(1, NCORES):
        acc += results[c]["out"].astype(np.float64)
    return acc.astype(np.float32).reshape(B, S, D)


def kernel(x, w_q, w_k, w_v, w_o):
    in_maps = make_in_maps(x, w_q, w_k, w_v, w_o)
    nc = _get_nc()
    res = run_bass_kernel_spmd(nc, in_maps, core_ids=list(range(NCORES)))
    return combine_outputs(res.results)


if __name__ == "__main__":
    rng = np.random.default_rng(0)
    ins = {
        "x": rng.standard_normal((B, S, D), dtype=np.float32),
        "w_q": rng.standard_normal((D, D), dtype=np.float32) * 0.02,
        "w_k": rng.standard_normal((D, D), dtype=np.float32) * 0.02,
        "w_v": rng.standard_normal((D, D), dtype=np.float32) * 0.02,
        "w_o": rng.standard_normal((D, D), dtype=np.float32) * 0.02,
    }
    y = kernel(**ins)
    print("kernel output", y.shape, y.dtype, float(np.abs(y).max()))


# revision 65
# speedup vs baseline: 1.0325x; 1.0146x over previous
"""Causal multi-head self-attention (QK-RMSNorm + tanh softcap) on 8 trn2 cores.

Problem (hardcoded): x [2, 2048, 1024], w_q/w_k/w_v/w_o [1024, 1024] fp32,
H=16 heads, dk=64, softcap 50, causal, out = softmax-attn @ w_o.T.

Sharding: head-parallel. Core c owns heads {2c, 2c+1} (128 local dims):
  - w_q/w_k/w_v sliced by rows -> per-core [128, 1024]; host pre-transposes.
  - w_o sliced by columns -> per-core [1024, 128]; host pre-transposes.
  - x is replicated (host pre-transposed to xT [1024, 4096], bf16).
  - Each core emits a full-shape bf16 partial output [4096, 1024]; host sums.

Numerics: the tanh softcap is dropped (|logits| <= 8 by Cauchy-Schwarz after
QK RMS norm, so tanh(s/50)*50 ~ s to ~2e-3 relative; measured end-to-end
error vs the fp32 reference is ~3.5e-3 against a 2e-2 gate, including the
bf16 input/output quantization). exp needs no running max (logits bounded).

On-core pipeline per 512-token tile (matmuls keyed on the bf16/f32r moving
operand run at 1 cycle/row):
  A) q/k: ps = wT.T @ x (8 k-chunk matmuls, bf16 in, f32 PSUM), staged to
     resident f32r qTn/kTn; per-head sumsq via eye2 matmul of sq = ACT
     Square(ps); rstd = 1/sqrt(ss) via a packed quake-Newton rsqrt
     (PE-transpose [2,512] rows into [128,16], seed on DVE, Newton iters on
     the otherwise-idle Pool engine, transpose back); the x8 RMS factor is
     folded into the eyeT8q/k broadcast matmuls; in-place normalize on DVE.
     v: projected directly into [token, dim] layout by 32 small matmuls
     (lhsT = x chunk), staged into vaug with ones columns for the softmax
     denominator. Fronts (through quake) and backs (unpack + normalize)
     are software-pipelined one tile apart.
  B) per (batch, head, 512-query block): scoresT[j, i] = k.T @ q blocks into
     [128, 1024] PSUM, exp straight off PSUM (scale 1/8) into f32r u tiles
     (diagonal trapezoid first so Pool mask latency stays off the PV tail),
     causal via block skip + triangular mask multiply (Pool), PV accumulate
     with v stationary (denominator rides along as row 64), normalize via
     reciprocal + K=1 ones matmul broadcast + DVE multiply. Output
     projections lag their attention core by one block.
  C) out[t, :] = ytt.T @ w_oT per 128-token block; PSUM staged to a bf16
     [128, 4, 1024] tile (DVE, with ACT taking a share; alternating on the
     drain blocks) and DMAd per 128-token row block.
"""

import sys

for _p in ("/opt/trn_rl_repo",):
    if _p not in sys.path:
        sys.path.insert(0, _p)

import numpy as np

import concourse.bacc as bacc
import concourse.tile as tile
from concourse import mybir
from concourse.bass_utils import run_bass_kernel_spmd

F32 = mybir.dt.float32
F32R = mybir.dt.float32r
BF16 = mybir.dt.bfloat16
AF = mybir.ActivationFunctionType
ALU = mybir.AluOpType

B, S, D = 2, 2048, 1024
H, DK = 16, 64
NCORES = 8
HLOC = H // NCORES          # 2 heads per core
MLOC = HLOC * DK            # 128 local head dims
T = B * S                   # 4096 tokens

TT = 512                    # token tile (phase A, also query i-block)
NTB = S // TT               # 4 token tiles per batch
JB = 128                    # key j-block
NJB = S // JB               # 16 j-blocks per batch
ISQDK = 1.0 / 8.0           # 1/sqrt(64)


def build_kernel():
    nc = bacc.Bacc("TRN2", target_bir_lowering=False, debug=False)

    xT = nc.dram_tensor("xT", [D, T], BF16, kind="ExternalInput")
    wqT = nc.dram_tensor("wqT", [D, MLOC], BF16, kind="ExternalInput")
    wkT = nc.dram_tensor("wkT", [D, MLOC], BF16, kind="ExternalInput")
    wvT = nc.dram_tensor("wvT", [D, MLOC], BF16, kind="ExternalInput")
    woT = nc.dram_tensor("woT", [MLOC, D], BF16, kind="ExternalInput")
    out = nc.dram_tensor("out", [T, D], BF16, kind="ExternalOutput")

    xT3 = xT.ap().rearrange("(o p) t -> p o t", p=128)      # [128, 8, 4096]
    out3 = out.ap().rearrange("(blk p) d -> p blk d", p=128)  # [128, 32, 1024]

    with tile.TileContext(nc) as tc:
        _emit(nc, tc, xT3, wqT, wkT, wvT, woT, out3)

    nc.compile()
    return nc


def _emit(nc, tc, xT3, wqT, wkT, wvT, woT, out3):
    from contextlib import ExitStack

    ctx = ExitStack()
    with ctx:
        cn = ctx.enter_context(tc.tile_pool(name="cn", bufs=1))
        # small staging tiles for the packed quake rsqrt chain
        nwt = ctx.enter_context(tc.tile_pool(name="nwt", bufs=8))
        xload = ctx.enter_context(tc.tile_pool(name="xload", bufs=3))
        wpool = ctx.enter_context(tc.tile_pool(name="wpool", bufs=1))
        qk_res = ctx.enter_context(tc.tile_pool(name="qk_res", bufs=1))
        sqp = ctx.enter_context(tc.tile_pool(name="sqp", bufs=3))
        rsp = ctx.enter_context(tc.tile_pool(name="rsp", bufs=4))
        up = ctx.enter_context(tc.tile_pool(name="up", bufs=6))
        ytn = ctx.enter_context(tc.tile_pool(name="ytn", bufs=3))
        osp = ctx.enter_context(tc.tile_pool(name="osp", bufs=2))
        # PSUM: sc 2x[128,1024] (4 banks) + yt 2x[65,512] (2) + mm 2x[128,512]
        # (2) = 8 banks. The whole rstd chain runs through PE transposes and
        # engine ops (no DMA hops), so its mm-slot reuse drains fast enough
        # to share the single mm tag with proj/outproj tiles.
        scp = ctx.enter_context(tc.tile_pool(name="scp", bufs=2, space="PSUM"))
        ytp = ctx.enter_context(tc.tile_pool(name="ytp", bufs=2, space="PSUM"))
        mmp = ctx.enter_context(tc.tile_pool(name="mmp", bufs=2, space="PSUM"))

        # ---- constants ----
        def rounded(name, f32_tile):
            t = cn.tile(list(f32_tile.shape), F32R, tag=name + "_r", name=name + "_r")
            nc.vector.tensor_copy(t, f32_tile)
            return t

        eye2_f = cn.tile([128, 2], F32, tag="eye2_f")
        nc.vector.memset(eye2_f, 0.0)
        nc.vector.memset(eye2_f[0:64, 0:1], 1.0)
        nc.vector.memset(eye2_f[64:128, 1:2], 1.0)
        eye2 = rounded("eye2", eye2_f)

        ident = cn.tile([128, 128], F32, tag="ident")
        nc.vector.memset(ident, 1.0)
        nc.gpsimd.affine_select(
            out=ident, in_=ident, pattern=[[1, 128]],
            compare_op=ALU.is_equal, fill=0.0, base=0, channel_multiplier=-1,
        )

        ident4 = cn.tile([4, 4], F32, tag="ident4")
        nc.vector.memset(ident4, 1.0)
        nc.gpsimd.affine_select(
            out=ident4, in_=ident4, pattern=[[1, 4]],
            compare_op=ALU.is_equal, fill=0.0, base=0, channel_multiplier=-1,
        )

        # eyeT8q[p, f] = 8 iff 0 <= f - 64p < 64 (rows 2,3 fall out as zero):
        # broadcasts rt rows 0:2 (q rstd per head) across their 64 partitions,
        # folding in the x8 RMS factor (rstd = 1/sqrt(sumsq); rms norm needs
        # 8/sqrt(sumsq)). eyeT8k does the same for rt rows 2:4 (k rstd).
        eyeT8q_f = cn.tile([4, 128], F32, tag="eyeT8q_f")
        nc.vector.memset(eyeT8q_f, 8.0)
        nc.gpsimd.affine_select(
            out=eyeT8q_f, in_=eyeT8q_f, pattern=[[1, 128]],
            compare_op=ALU.is_ge, fill=0.0, base=0, channel_multiplier=-64,
        )
        nc.gpsimd.affine_select(
            out=eyeT8q_f, in_=eyeT8q_f, pattern=[[-1, 128]],
            compare_op=ALU.is_ge, fill=0.0, base=63, channel_multiplier=64,
        )
        eyeT8q = rounded("eyeT8q", eyeT8q_f)

        eyeT8k_f = cn.tile([4, 128], F32, tag="eyeT8k_f")
        nc.vector.memset(eyeT8k_f, 8.0)
        nc.gpsimd.affine_select(
            out=eyeT8k_f, in_=eyeT8k_f, pattern=[[1, 128]],
            compare_op=ALU.is_ge, fill=0.0, base=128, channel_multiplier=-64,
        )
        nc.gpsimd.affine_select(
            out=eyeT8k_f, in_=eyeT8k_f, pattern=[[-1, 128]],
            compare_op=ALU.is_ge, fill=0.0, base=-65, channel_multiplier=64,
        )
        eyeT8k = rounded("eyeT8k", eyeT8k_f)

        ones1_f = cn.tile([1, 64], F32, tag="ones1_f")
        nc.vector.memset(ones1_f, 1.0)
        ones1 = rounded("ones1", ones1_f)

        # tri128[p, f] = 1 if f >= p else 0  (keep i>=j in [j, i] tiles)
        tri_f = cn.tile([128, 128], F32, tag="tri_f")
        nc.vector.memset(tri_f, 1.0)
        nc.gpsimd.affine_select(
            out=tri_f, in_=tri_f, pattern=[[1, 128]],
            compare_op=ALU.is_ge, fill=0.0, base=0, channel_multiplier=-1,
        )
        tri = rounded("tri", tri_f)

        # dtri256[p, f] = 1 if f - 128 >= p else 0 (s=3 diag tile, 256 wide)
        dtri_f = cn.tile([128, 256], F32, tag="dtri_f")
        nc.vector.memset(dtri_f, 1.0)
        nc.gpsimd.affine_select(
            out=dtri_f, in_=dtri_f, pattern=[[1, 256]],
            compare_op=ALU.is_ge, fill=0.0, base=-128, channel_multiplier=-1,
        )
        dtri = rounded("dtri", dtri_f)

        ones16 = cn.tile([128, 16], F32, tag="ones16")
        nc.vector.memset(ones16, 1.0)

        # first x tile loads ahead of the weights so the first projection
        # matmuls are never waiting on the DMA queue
        xt00 = xload.tile([128, 8, TT], BF16, tag="xt")
        nc.sync.dma_start(out=xt00[:, 0:4, :], in_=xT3[:, 0:4, 0:TT])
        wq_t = wpool.tile([128, 8, MLOC], BF16, tag="wq")
        nc.sync.dma_start(out=wq_t, in_=wqT.ap().rearrange("(o p) m -> p o m", p=128))
        nc.sync.dma_start(out=xt00[:, 4:8, :], in_=xT3[:, 4:8, 0:TT])
        wk_t = wpool.tile([128, 8, MLOC], BF16, tag="wk")
        nc.sync.dma_start(out=wk_t, in_=wkT.ap().rearrange("(o p) m -> p o m", p=128))
        wv_t = wpool.tile([128, 8, MLOC], BF16, tag="wv")
        nc.sync.dma_start(out=wv_t, in_=wvT.ap().rearrange("(o p) m -> p o m", p=128))

        wo_t = wpool.tile([128, D], BF16, tag="wo")
        nc.sync.dma_start(out=wo_t, in_=woT.ap())

        # ---- residents ----
        qTn = [qk_res.tile([128, S], F32R, tag=f"qTn{b}", name=f"qTn{b}") for b in range(B)]
        kTn = [qk_res.tile([128, S], F32R, tag=f"kTn{b}", name=f"kTn{b}") for b in range(B)]
        # v_aug[b]: [128(t within j-block), jb, 130] = [v_h0 | 1 | v_h1 | 1]
        vaug = [qk_res.tile([128, NJB, 130], F32R, tag=f"vaug{b}", name=f"vaug{b}") for b in range(B)]
        for b in range(B):
            nc.vector.tensor_copy(vaug[b][:, :, 64], ones16)
            nc.vector.tensor_copy(vaug[b][:, :, 129], ones16)

        # ---- phase A: one 512-token tile of projections + norms ----
        # phase A is software-pipelined as front/back halves: the front ends
        # at the packed quake rsqrt (DVE); the back picks up with the unpack
        # transposes. Emitting front(t+1) between them gives PE independent
        # matmul work to chew while the DVE chain of tile t drains.
        def load_x(b, tt):
            t0g = b * S + tt * TT
            xt = xload.tile([128, 8, TT], BF16, tag="xt")
            nc.sync.dma_start(out=xt[:, 0:4, :], in_=xT3[:, 0:4, t0g : t0g + TT])
            nc.sync.dma_start(out=xt[:, 4:8, :], in_=xT3[:, 4:8, t0g : t0g + TT])
            return xt

        def phase_a_front(b, tt, xt=None):
            t0 = tt * TT
            if xt is None:
                xt = load_x(b, tt)
            # q/k projections first: their PSUM tiles drain quickly (DVE
            # staging copy + ACT square run in parallel straight off PSUM)
            sqs = {}
            for w_t, which in ((wq_t, "q"), (wk_t, "k")):
                dest = qTn[b] if which == "q" else kTn[b]
                sl = dest[:, t0 : t0 + TT]
                ps = mmp.tile([128, TT], F32, tag="mm")
                for k in range(8):
                    nc.tensor.matmul(
                        ps, w_t[:, k, :], xt[:, k, :],
                        start=(k == 0), stop=(k == 7)
                    )
                if b == 0:
                    nc.scalar.copy(sl, ps)
                else:
                    nc.vector.tensor_copy(sl, ps)
                sq = sqp.tile([128, TT], F32R, tag="sq")
                if b == 0:
                    nc.scalar.square(sq, ps)
                else:
                    # batch 1's squares run while ACT is busy with batch 0's
                    # attention exps; DVE has more slack in that window
                    nc.vector.tensor_mul(sq, sl, sl)
                del ps
                sqs[which] = sq
            # v: project straight into [token, dim] layout (lhsT = x chunk)
            vt = mmp.tile([128, TT], F32, tag="mm")
            for sub in range(4):
                c0 = 128 * sub
                for k in range(8):
                    nc.tensor.matmul(
                        vt[:, c0 : c0 + 128],
                        xt[:, k, c0 : c0 + 128],
                        wv_t[:, k, :],
                        start=(k == 0), stop=(k == 7)
                    )
            for sub in range(4):
                jb = tt * 4 + sub
                c0 = 128 * sub
                nc.vector.tensor_copy(
                    vaug[b][:, jb, 0:130].rearrange(
                        "p (two c) -> p two c", two=2)[:, :, 0:64],
                    vt[:, c0 : c0 + 128].rearrange(
                        "p (two c) -> p two c", two=2),
                )
            del vt, xt
            # stage per-head sumsq rows to SBUF (ACT Copy is in every act
            # table, so this staging costs no table switch); separate q/k
            # tiles keep every partition start at 0 (32-align rule)
            sts = {}
            for which in ("q", "k"):
                ss = mmp.tile([2, TT], F32, tag="mm")
                nc.tensor.matmul(ss, eye2, sqs[which], start=True, stop=True)
                stw = nwt.tile([2, TT], F32, tag="st" + which,
                               name=f"st{which}{b}_{tt}")
                if b == 0:
                    nc.scalar.copy(stw, ss)
                else:
                    nc.vector.tensor_copy(stw, ss)
                sts[which] = stw
                del ss
            del sqs
            # packed quake rsqrt: PE-transpose the [2, 512] rows into
            # [128, 16] so the DVE Newton ops run on 16-wide rows instead of
            # 512-wide ones (transposes instead of DMAs keep the chain
            # latency off the PE critical path). ssP cols 4c+{0,1} = q heads,
            # 4c+{2,3} = k heads for token chunk c.
            ssP = mmp.tile([128, 16], F32, tag="mm")
            for c in range(4):
                nc.tensor.transpose(
                    ssP[:, 4 * c : 4 * c + 2],
                    sts["q"][:, 128 * c : 128 * c + 128],
                    ident4[0:2, 0:2],
                )
                nc.tensor.transpose(
                    ssP[:, 4 * c + 2 : 4 * c + 4],
                    sts["k"][:, 128 * c : 128 * c + 128],
                    ident4[0:2, 0:2],
                )
            del sts
            y = nwt.tile([128, 16], F32, tag="nwt_y", name=f"y{b}_{tt}")
            t1 = nwt.tile([128, 16], F32, tag="nwt_t", name=f"t{b}_{tt}")
            v0 = nwt.tile([128, 16], F32, tag="nwt_v", name=f"v{b}_{tt}")
            # seed + ssP staging on DVE (Pool cannot read PSUM), Newton on
            # the otherwise-idle Pool engine so the chain does not queue
            # behind DVE's staging copies
            nc.vector.tensor_copy(v0, ssP)
            nc.vector.tensor_scalar(
                y.bitcast(mybir.dt.int32), ssP.bitcast(mybir.dt.int32),
                1, None, ALU.logical_shift_right,
            )
            del ssP
            nc.gpsimd.tensor_scalar(
                y.bitcast(mybir.dt.int32), y.bitcast(mybir.dt.int32),
                -1, 0x5F3759DF, ALU.mult, ALU.add,
            )
            # 2 Newton iterations: rel err ~4e-6, far below the bf16 input
            # quantization already accepted
            for _ in range(2):
                nc.gpsimd.tensor_mul(t1, y, y)
                nc.gpsimd.tensor_mul(t1, t1, v0)
                nc.gpsimd.tensor_scalar(t1, t1, -0.5, 1.5, ALU.mult, ALU.add)
                nc.gpsimd.tensor_mul(y, y, t1)
            del t1, v0
            return y

        def phase_a_back(b, tt, y):
            t0 = tt * TT
            # unpack back to [4, 512] rows via transposes, one staging copy
            rtP = mmp.tile([4, TT], F32, tag="mm")
            for c in range(4):
                nc.tensor.transpose(
                    rtP[:, 128 * c : 128 * c + 128], y[:, 4 * c : 4 * c + 4],
                    ident,
                )
            del y
            rt = nwt.tile([4, TT], F32R, tag="rt", name=f"rt{b}_{tt}")
            nc.scalar.copy(rt, rtP)
            del rtP
            for which, eyeT8 in (("q", eyeT8q), ("k", eyeT8k)):
                dest = qTn[b] if which == "q" else kTn[b]
                sl = dest[:, t0 : t0 + TT]
                bc = mmp.tile([128, TT], F32, tag="mm")
                nc.tensor.matmul(bc, eyeT8, rt, start=True, stop=True)
                nc.vector.tensor_mul(sl, sl, bc)
                del bc
            del rt

        # ---- phase B/C: attention + output projection ----
        def qk(sc_slice, b, h, jbl, i0, iw):
            """scoresT[j, i] block: lhsT = kT [64, 128] (j), rhs = qT [64, iw]."""
            nc.tensor.matmul(
                sc_slice,
                kTn[b][64 * h : 64 * h + 64, 128 * jbl : 128 * jbl + 128],
                qTn[b][64 * h : 64 * h + 64, i0 : i0 + iw],
                start=True,
                stop=True,
            )

        def pv(yt, b, h, jbl, u_slice, icol, first, last):
            nc.tensor.matmul(
                yt[:, icol : icol + u_slice.shape[-1]],
                vaug[b][:, jbl, 65 * h : 65 * h + 65],
                u_slice,
                start=first,
                stop=last,
            )

        def attn_core(b, n):
            i0 = n * TT
            ytt = ytn.tile([128, TT], BF16, tag="ytt")
            for h in range(HLOC):
                yt = ytp.tile([65, TT], F32, tag="yt")
                # diagonal first: 4 j-blocks, trapezoid widths + triangular
                # masks. Leading with them keeps the Pool mask latency off
                # the tail of the PV accumulation chain.
                # u1 cols: [0:512]@i0 (jb0), [512:896]@i0+128 (jb0+1)
                # u2 cols: [0:256]@i0+256 (jb0+2), [256:512]@i0+256 (jb0+3)
                jb0 = 4 * n
                sc = scp.tile([128, 1024], F32, tag="sc")
                qk(sc[:, 0:512], b, h, jb0, i0, 512)
                qk(sc[:, 512:896], b, h, jb0 + 1, i0 + 128, 384)
                u1 = up.tile([128, 1024], F32R, tag="u")
                nc.scalar.activation(u1[:, 0:896], sc[:, 0:896], AF.Exp,
                                     scale=ISQDK)
                del sc
                sc = scp.tile([128, 1024], F32, tag="sc")
                qk(sc[:, 0:256], b, h, jb0 + 2, i0 + 256, 256)
                qk(sc[:, 256:512], b, h, jb0 + 3, i0 + 256, 256)
                u2 = up.tile([128, 1024], F32R, tag="u")
                nc.scalar.activation(u2[:, 0:512], sc[:, 0:512], AF.Exp,
                                     scale=ISQDK)
                del sc
                nc.gpsimd.tensor_mul(u1[:, 0:128], u1[:, 0:128], tri)
                nc.gpsimd.tensor_mul(u1[:, 512:640], u1[:, 512:640], tri)
                nc.gpsimd.tensor_mul(u2[:, 0:128], u2[:, 0:128], tri)
                nc.gpsimd.tensor_mul(u2[:, 256:512], u2[:, 256:512], dtri)
                # full j-blocks below the diagonal, two per scores tile; the
                # diag PVs run LAST so the Pool mask latency is hidden under
                # the off-diag stream
                for jp in range(2 * n):
                    jbl = 2 * jp
                    sc = scp.tile([128, 1024], F32, tag="sc")
                    qk(sc[:, 0:512], b, h, jbl, i0, 512)
                    qk(sc[:, 512:1024], b, h, jbl + 1, i0, 512)
                    u = up.tile([128, 1024], F32R, tag="u")
                    nc.scalar.activation(u, sc, AF.Exp, scale=ISQDK)
                    del sc
                    pv(yt, b, h, jbl, u[:, 0:512], 0, first=(jp == 0),
                       last=False)
                    pv(yt, b, h, jbl + 1, u[:, 512:1024], 0, first=False,
                       last=False)
                    del u
                pv(yt, b, h, jb0, u1[:, 0:512], 0, first=(n == 0), last=False)
                pv(yt, b, h, jb0 + 1, u1[:, 512:896], 128, first=False,
                   last=False)
                del u1
                pv(yt, b, h, jb0 + 2, u2[:, 0:256], 256, first=False,
                   last=False)
                pv(yt, b, h, jb0 + 3, u2[:, 256:512], 256, first=False,
                   last=True)
                del u2

                # normalize this head: rden = 1/denominator, broadcast via
                # K=1 matmul, stage yt to SBUF (one-PSUM-input rule), multiply
                rden = rsp.tile([1, TT], F32R, tag="rden")
                with nc.allow_low_precision(reason="fp32r matmul operand"):
                    nc.vector.reciprocal(rden, yt[64:65, :])
                bc2 = mmp.tile([64, TT], F32, tag="mm")
                nc.tensor.matmul(bc2, ones1, rden, start=True, stop=True)
                del rden
                ytsb = ytn.tile([64, TT], F32, tag="ytsb")
                nc.vector.tensor_copy(ytsb, yt[0:64, :])
                del yt
                if h == 0:
                    nc.vector.tensor_mul(ytt[0:64, :], ytsb, bc2)
                else:
                    y1 = ytn.tile([64, TT], BF16, tag="y1")
                    nc.vector.tensor_mul(y1, ytsb, bc2)
                    # partition shift 0..63 -> 64..127 via SBUF-to-SBUF DMA
                    nc.sync.dma_start(out=ytt[64:128, :], in_=y1)
                    del y1
                del ytsb, bc2
            return ytt

        def attn_out(b, n, ytt, drain=False):
            # phase C: out[t, :] = ytt.T @ woT, 128-token sub-blocks, staged
            # to a bf16 tile; per-ts DMAs start draining as soon as each
            # 128-token row block is converted. drain=True alternates the
            # staging copies across DVE/ACT so the kernel tail is not paced
            # by a single engine.
            os = osp.tile([128, 4, D], BF16, tag="os")
            blk0 = b * (S // 128) + 4 * n
            for ts in range(4):
                for nn in range(2):
                    op = mmp.tile([128, 512], F32, tag="mm")
                    nc.tensor.matmul(
                        op,
                        ytt[:, 128 * ts : 128 * ts + 128],
                        wo_t[:, 512 * nn : 512 * nn + 512],
                        start=True,
                        stop=True,
                    )
                    dst = os[:, ts, 512 * nn : 512 * nn + 512]
                    on_act = False if not drain else ((2 * ts + nn) % 2 == 1)
                    if on_act:
                        nc.scalar.copy(dst, op)
                    else:
                        nc.vector.tensor_copy(dst, op)
                    del op
                nc.sync.dma_start(
                    out=out3[:, blk0 + ts : blk0 + ts + 1, :],
                    in_=os[:, ts : ts + 1, :],
                )
            del ytt, os

        # ---- emission: software-pipelined. Phase-A fronts run one tile
        # ahead of backs; attention cores run one block ahead of their
        # output projections; phase A(b1) threads between attention(b0)
        # blocks so PE always has independent matmul work queued. ----
        ys = {}
        ys[(0, 0)] = phase_a_front(0, 0, xt=xt00)
        for tt in range(1, NTB):
            ys[(0, tt)] = phase_a_front(0, tt)
            phase_a_back(0, tt - 1, ys.pop((0, tt - 1)))
        phase_a_back(0, NTB - 1, ys.pop((0, NTB - 1)))

        ytt_prev = None  # (b, n, ytt) awaiting its output projection
        border = (3, 2, 1, 0)
        for i, n in enumerate(range(NTB)):
            ytt = attn_core(0, n)
            if ytt_prev is not None:
                attn_out(*ytt_prev)
            ytt_prev = (0, n, ytt)
            ys[(1, i)] = phase_a_front(1, i)
            if i > 0:
                phase_a_back(1, i - 1, ys.pop((1, i - 1)))
        phase_a_back(1, NTB - 1, ys.pop((1, NTB - 1)))
        for n in range(NTB):
            ytt = attn_core(1, n)
            attn_out(*ytt_prev, drain=(n == NTB - 1))
            ytt_prev = (1, n, ytt)
        attn_out(*ytt_prev, drain=True)


_NC_CACHE = None


def _get_nc():
    global _NC_CACHE
    if _NC_CACHE is None:
        _NC_CACHE = build_kernel()
    return _NC_CACHE


def make_in_maps(x, w_q, w_k, w_v, w_o):
    import ml_dtypes

    bf16 = ml_dtypes.bfloat16
    x = np.ascontiguousarray(np.asarray(x, dtype=np.float32))
    w_q = np.asarray(w_q, dtype=np.float32)
    w_k = np.asarray(w_k, dtype=np.float32)
    w_v = np.asarray(w_v, dtype=np.float32)
    w_o = np.asarray(w_o, dtype=np.float32)

    xT = np.ascontiguousarray(x.reshape(T, D).T).astype(bf16)  # [D, T]
    in_maps = []
    for c in range(NCORES):
        hs = slice(c * MLOC, (c + 1) * MLOC)
        in_maps.append(
            {
                "xT": xT,
                "wqT": np.ascontiguousarray(w_q[hs, :].T).astype(bf16),
                "wkT": np.ascontiguousarray(w_k[hs, :].T).astype(bf16),
                "wvT": np.ascontiguousarray(w_v[hs, :].T).astype(bf16),
                "woT": np.ascontiguousarray(w_o[:, hs].T).astype(bf16),
            }
        )
    return in_maps


def combine_outputs(results):
    acc = results[0]["out"].astype(np.float64)
    for c in range(1, NCORES):
        acc += results[c]["out"].astype(np.float64)
    return acc.astype(np.float32).reshape(B, S, D)


def kernel(x, w_q, w_k, w_v, w_o):
    in_maps = make_in_maps(x, w_q, w_k, w_v, w_o)
    nc = _get_nc()
    res = run_bass_kernel_spmd(nc, in_maps, core_ids=list(range(NCORES)))
    return combine_outputs(res.results)


if __name__ == "__main__":
    rng = np.random.default_rng(0)
    ins = {
        "x": rng.standard_normal((B, S, D), dtype=np.float32),
        "w_q": rng.standard_normal((D, D), dtype=np.float32) * 0.02,
        "w_k": rng.standard_normal((D, D), dtype=np.float32) * 0.02,
        "w_v": rng.standard_normal((D, D), dtype=np.float32) * 0.02,
        "w_o": rng.standard_normal((D, D), dtype=np.float32) * 0.02,
    }
    y = kernel(**ins)
    print("kernel output", y.shape, y.dtype, float(np.abs(y).max()))
